# revision 10
# baseline (speedup 1.0000x reference)
"""Trainium2 Bass kernel for EnhancedCondConv2d (moe_routing).

Data-parallel over batch: 8 cores x 2 samples each. Full inputs in,
full outputs back.

Per-core program:
  1. routing (both samples): avgpool(x) -> tiny MLP -> softmax
  2. wgen (both samples fused): w[b] = sum_e rw[e]*experts[e] as 144
     block-diag matmuls with contiguous (FWL-friendly) expert layout
  3. conv per sample: 9 PSUM-accumulated shifted matmuls per 8-row
     strip, double-buffered PSUM so PE never stalls on eviction
  4. SE channel attention folded into PSUM eviction accumulators
  5. CBAM stats: cw-scaled transpose of each h-plane via one matmul
     against diag(cw) -> DVE max-reduce gives the channel max in
     [w-part, h] layout; channel mean via cw-weighted column matmul in
     flat [h, w] layout. 7x7 conv as banded-Toeplitz matmuls in both
     layouts, folded into one PSUM via an identity matmul. Sample 0's
     stats interleave into sample 1's conv emission so PE stays dense.
  6. final: out = (osb*cw)*sw + x in place, all bf16 (host casts f32)

Perf notes: head DMAs merged to 8 so the 8 DMA sem lanes never block;
sigmoids via Exp+reciprocal and ReLU on DVE so ACT only ever loads the
Copy/Exp tables once.
"""

import math
from contextlib import ExitStack

import numpy as np

import concourse.bass as bass
import concourse.bacc as bacc
import concourse.mybir as mybir
import concourse.tile as tile
from concourse.bass_utils import run_bass_kernel_spmd

F32 = mybir.dt.float32
BF16 = mybir.dt.bfloat16
AX = mybir.AxisListType
ALU = mybir.AluOpType
ACTF = mybir.ActivationFunctionType

B, CI, CO, H, W, E, RR = 16, 128, 128, 128, 128, 16, 8
NCORES = 8
BL = B // NCORES  # 2 samples per core
EPS = 1e-5
HW = H * W
BNS = 1.0 / math.sqrt(1.0 + EPS)

_CACHE = {}


def _build_module():
    nc = bacc.Bacc("TRN2", target_bir_lowering=False, debug=False)

    xp_d = nc.dram_tensor("xpad", [BL, CI, H + 2, W + 2], BF16,
                          kind="ExternalInput").ap()
    ew_d = nc.dram_tensor("experts_w", [4, 128, 4, 9, 128], BF16,
                          kind="ExternalInput").ap()
    pkf_d = nc.dram_tensor("packf", [128, 330], F32, kind="ExternalInput").ap()
    pkc_d = nc.dram_tensor("packc", [128, 1928], BF16,
                           kind="ExternalInput").ap()

    out_d = nc.dram_tensor("out", [BL, CO, H, W], BF16,
                           kind="ExternalOutput").ap()

    ssw_d = nc.dram_tensor("scr_sw", [BL, HW], BF16).ap()

    with tile.TileContext(nc) as tc, ExitStack() as ctx:
        _kernel_body(ctx, tc, xp_d, ew_d, pkf_d, pkc_d, out_d, ssw_d)
    nc.compile()
    return nc


def _kernel_body(ctx, tc, xp_d, ew_d, pkf_d, pkc_d, out_d, ssw_d):
    nc = tc.nc

    cpool = ctx.enter_context(tc.tile_pool(name="const", bufs=1))
    xpool = ctx.enter_context(tc.tile_pool(name="xp", bufs=1))
    opool = ctx.enter_context(tc.tile_pool(name="ob", bufs=2))
    wpool = ctx.enter_context(tc.tile_pool(name="wp", bufs=1))
    epool = ctx.enter_context(tc.tile_pool(name="ep", bufs=2))
    spool = ctx.enter_context(tc.tile_pool(name="sp", bufs=2))
    fpool = ctx.enter_context(tc.tile_pool(name="fp", bufs=2))
    f1pool = ctx.enter_context(tc.tile_pool(name="f1", bufs=1))

    pc = ctx.enter_context(tc.tile_pool(name="pc", bufs=4, space="PSUM"))
    pm = ctx.enter_context(tc.tile_pool(name="pm", bufs=4, space="PSUM"))

    # ---------- bulk loads: 8 DMAs total, sync ring, priority order ----------
    xp0 = xpool.tile([128, H + 2, W + 2], BF16, tag="xp0")
    nc.sync.dma_start(xp0, xp_d[0])
    pkf = cpool.tile([128, 330], F32, tag="pkf")
    nc.sync.dma_start(pkf, pkf_d)
    xp1 = xpool.tile([128, H + 2, W + 2], BF16, tag="xp1")
    nc.sync.dma_start(xp1, xp_d[1])
    pkc = cpool.tile([128, 1928], BF16, tag="pkc")
    nc.sync.dma_start(pkc, pkc_d)
    xp = [xp0, xp1]
    ecs = []
    for g4 in range(4):
        ec = epool.tile([128, 4, 9, 128], BF16, tag="ec", name=f"ec{g4}")
        nc.sync.dma_start(ec, ew_d[g4])
        ecs.append(ec)

    one11 = cpool.tile([1, 1], F32, tag="one11")
    nc.vector.memset(one11, 1.0)

    # const views
    rw1t = pkf[:, 0:16]
    rw3t = pkf[:, 16:32]
    caw1t = pkf[:, 32:48]
    gs2n = pkf[:, 48:49]
    bb2n = pkf[:, 49:50]
    gsca2n = pkf[:, 50:51]
    bbca2n = pkf[:, 51:52]
    gssan = pkf[:, 52:53]
    bssan = pkf[:, 53:54]
    rw2t = pkf[0:16, 54:182]
    caw2t = pkf[0:16, 182:310]
    gs1 = pkf[0:16, 310:311]
    bb1 = pkf[0:16, 311:312]
    gsca1 = pkf[0:16, 312:313]
    bbca1 = pkf[0:16, 313:314]
    rb3r = pkf[0:1, 314:330]
    msum = pkc[:, 0:896].rearrange("p (t i) -> p t i", t=7)
    mmax = pkc[:, 896:1792].rearrange("p (t i) -> p t i", t=7)
    identb = pkc[:, 1792:1920]
    bmask = pkc[:, 1920:1928]

    # ---------- routing (both samples) ----------
    rwcols = []
    for b in range(BL):
        psA = spool.tile([128, 1], F32, tag="psA")
        nc.vector.tensor_reduce(psA, xp[b][:, 0:64, :], AX.XY, ALU.add)
        pparts = spool.tile([128, 8], F32, tag="pparts")
        for i in range(6):
            r0 = 64 + 11 * i
            pscr = f1pool.tile([128, 11, W + 2], BF16, tag="pscr")
            nc.scalar.activation(
                pscr, xp[b][:, r0:r0 + 11, :], ACTF.Copy,
                accum_out=pparts[:, i:i + 1])
        nc.vector.memset(pparts[:, 6:8], 0.0)
        psB = spool.tile([128, 1], F32, tag="psB")
        nc.vector.tensor_reduce(psB, pparts, AX.X, ALU.add)
        psum_t = spool.tile([128, 1], F32, tag="psum_t")
        nc.vector.tensor_add(psum_t, psA, psB)

        mm1 = pm.tile([16, 1], F32, tag="m")
        nc.tensor.matmul(mm1, rw1t, psum_t, start=True, stop=True)
        h1 = spool.tile([16, 1], F32, tag="h1")
        nc.vector.scalar_tensor_tensor(h1, mm1, gs1, bb1, ALU.mult, ALU.add)
        nc.vector.tensor_scalar_max(h1, h1, 0.0)
        mm2 = pm.tile([128, 1], F32, tag="m")
        nc.tensor.matmul(mm2, rw2t, h1, start=True, stop=True)
        gg = spool.tile([128, 1], F32, tag="gg")
        nc.scalar.activation(gg, mm2, ACTF.Exp, bias=bb2n, scale=gs2n)
        nc.vector.tensor_scalar_add(gg, gg, 1.0)
        nc.vector.reciprocal(gg, gg)
        mm3 = pm.tile([1, E], F32, tag="m")
        nc.tensor.matmul(mm3, gg, rw3t, start=True, stop=True)
        lg = spool.tile([1, E], F32, tag="lg")
        nc.vector.tensor_add(lg, mm3, rb3r)
        mx = spool.tile([1, 1], F32, tag="mx")
        nc.vector.tensor_reduce(mx, lg, AX.X, ALU.max)
        mxn = spool.tile([1, 1], F32, tag="mxn")
        nc.vector.tensor_scalar_mul(mxn, mx, -1.0)
        e16 = spool.tile([1, E], F32, tag="e16")
        nc.scalar.activation(e16, lg, ACTF.Exp, bias=mxn, scale=1.0)
        s1 = spool.tile([1, 1], F32, tag="s1")
        nc.vector.tensor_reduce(s1, e16, AX.X, ALU.add)
        rinv = spool.tile([1, 1], F32, tag="rinv")
        nc.vector.reciprocal(rinv, s1)
        e128 = spool.tile([1, 128], F32, tag="e128")
        nc.vector.tensor_scalar_mul(
            e128.rearrange("p (a c) -> p a c", a=8),
            e16.unsqueeze(1).broadcast_to([1, 8, E]), rinv)
        pcol = pm.tile([128, 1], F32, tag="m")
        nc.tensor.matmul(pcol, e128, one11, start=True, stop=True)
        rwcol = spool.tile([128, 1], F32, tag=f"rwcol{b}", name=f"rwcol{b}")
        nc.vector.tensor_copy(rwcol, pcol)
        rwcols.append(rwcol)

    rwblk = spool.tile([128, 2 * RR], BF16, tag="rwblk")
    nc.vector.tensor_scalar_mul(rwblk[:, 0:8], bmask, rwcols[0])
    nc.vector.tensor_scalar_mul(rwblk[:, 8:16], bmask, rwcols[1])

    # ---------- wgen (both samples) ----------
    wsb = [wpool.tile([128, 9, 128], BF16, tag=f"wsb{b}", name=f"wsb{b}")
           for b in range(BL)]
    for og in range(16):
        ec = ecs[og // 4]
        pw = pm.tile([128, 9, 16], F32, tag="m")
        for k in range(9):
            nc.tensor.matmul(pw[:, k, :], ec[:, og % 4, k, :], rwblk,
                             start=True, stop=True)
        if og % 2 == 0:
            nc.scalar.activation(wsb[0][:, :, og * 8:og * 8 + 8],
                                 pw[:, :, 0:8], ACTF.Copy)
            nc.vector.tensor_copy(wsb[1][:, :, og * 8:og * 8 + 8],
                                  pw[:, :, 8:16])
        else:
            nc.vector.tensor_copy(wsb[0][:, :, og * 8:og * 8 + 8],
                                  pw[:, :, 0:8])
            nc.scalar.activation(wsb[1][:, :, og * 8:og * 8 + 8],
                                 pw[:, :, 8:16], ACTF.Copy)

    # CBAM sp-map tiles: pads written once, reused across samples
    spsum_t = spool.tile([128, 134], BF16, tag="spsum_t")   # [h, w+pad]
    spmax_wh = spool.tile([128, 134], BF16, tag="spmax_wh")  # [w, h+pad]
    for t in (spsum_t, spmax_wh):
        nc.vector.memset(t[:, 0:3], 0.0)
        nc.vector.memset(t[:, 131:134], 0.0)

    # ---------- per-sample phases ----------
    osb = [opool.tile([128, H, W], BF16, tag="osb", name=f"osb{b}")
           for b in range(BL)]
    cparts = [spool.tile([128, 32], F32, tag="cparts", name=f"cparts{b}")
              for b in range(BL)]
    cw = [None, None]
    cwb = [None, None]
    diagcw = [None, None]

    def conv_strip(b, sup):
        pcs = [pc.tile([128, 4, W], F32, tag="c", name=f"pc{b}_{sup}_{g}")
               for g in range(2)]
        for k in range(9):
            kh, kw = divmod(k, 3)
            lhs = wsb[b][:, k, :]
            for g in range(2):
                r0 = sup * 8 + g * 4 + kh
                nc.tensor.matmul(pcs[g], lhs, xp[b][:, r0:r0 + 4, kw:kw + W],
                                 start=(k == 0), stop=(k == 8))
        for g in range(2):
            hr = sup * 8 + g * 4
            nc.scalar.activation(
                osb[b][:, hr:hr + 4, :], pcs[g], ACTF.Copy,
                accum_out=cparts[b][:, sup * 2 + g:sup * 2 + g + 1])

    def se_block(b):
        cps = spool.tile([128, 1], F32, tag="cps")
        nc.vector.tensor_reduce(cps, cparts[b], AX.X, ALU.add)
        se1 = pm.tile([16, 1], F32, tag="m")
        nc.tensor.matmul(se1, caw1t, cps, start=True, stop=True)
        chs = spool.tile([16, 1], F32, tag="chs")
        nc.vector.scalar_tensor_tensor(chs, se1, gsca1, bbca1,
                                       ALU.mult, ALU.add)
        nc.vector.tensor_scalar_max(chs, chs, 0.0)
        se2 = pm.tile([128, 1], F32, tag="m")
        nc.tensor.matmul(se2, caw2t, chs, start=True, stop=True)
        cw[b] = spool.tile([128, 1], F32, tag=f"cw{b}", name=f"cw{b}")
        nc.scalar.activation(cw[b], se2, ACTF.Exp, bias=bbca2n, scale=gsca2n)
        nc.vector.tensor_scalar_add(cw[b], cw[b], 1.0)
        nc.vector.reciprocal(cw[b], cw[b])
        cwb[b] = spool.tile([128, 1], BF16, tag=f"cwb{b}", name=f"cwb{b}")
        nc.vector.tensor_copy(cwb[b], cw[b])
        diagcw[b] = spool.tile([128, 128], BF16, tag=f"diagcw{b}",
                               name=f"diagcw{b}")
        nc.vector.tensor_scalar_mul(diagcw[b], identb, cw[b])

    def stats_chunk(b, c):
        mf = f1pool.tile([1, 4096], BF16, tag="mf")
        for j in range(8):
            h0 = c * 32 + j * 4
            ptt = pc.tile([128, 4, 128], F32, tag="c", name=f"ptt{b}_{c}_{j}")
            for i in range(4):
                nc.tensor.matmul(ptt[:, i, :], osb[b][:, h0 + i, :], diagcw[b],
                                 start=True, stop=True)
            pmean = pm.tile([1, 512], F32, tag="m")
            nc.tensor.matmul(pmean, cwb[b], osb[b][:, h0:h0 + 4, :],
                             start=True, stop=True)
            nc.vector.tensor_reduce(spmax_wh[:, 3 + h0:3 + h0 + 4], ptt,
                                    AX.X, ALU.max)
            nc.scalar.activation(mf[:, j * 512:(j + 1) * 512], pmean,
                                 ACTF.Copy)
        nc.sync.dma_start(spsum_t[c * 32:(c + 1) * 32, 3:131], mf)

    def banded_final(b):
        for c4 in range(4):
            sc = osb[b][:, c4 * 32:(c4 + 1) * 32, :]
            nc.vector.tensor_scalar_mul(sc, sc, cw[b])
        pswW = pm.tile([128, 128], F32, tag="m")
        for t in range(7):
            nc.tensor.matmul(pswW, mmax[:, t, :], spmax_wh[:, t:t + 128],
                             start=(t == 0), stop=(t == 6))
        swW = spool.tile([128, 128], BF16, tag="swW")
        nc.scalar.activation(swW, pswW, ACTF.Copy)
        psw = pm.tile([128, 128], F32, tag="m")
        for t in range(7):
            nc.tensor.matmul(psw, msum[:, t, :], spsum_t[:, t:t + 128],
                             start=(t == 0), stop=False)
        nc.tensor.matmul(psw, swW, identb, start=False, stop=True)
        swe = spool.tile([128, 128], F32, tag="swe")
        nc.scalar.activation(swe, psw, ACTF.Exp, bias=bssan, scale=gssan)
        nc.vector.tensor_scalar_add(swe, swe, 1.0)
        nc.vector.reciprocal(swe, swe)
        swsb = spool.tile([128, 128], BF16, tag="swsb")
        nc.vector.tensor_copy(swsb, swe)
        nc.sync.dma_start(ssw_d[b].rearrange("(h w) -> h w", h=128), swsb)

        for q in range(8):
            swbc = fpool.tile([128, 16, 128], BF16, tag="swbc")
            nc.sync.dma_start(
                swbc,
                ssw_d[b, q * 2048:(q + 1) * 2048].unsqueeze(0)
                .partition_broadcast(128))
            sl = osb[b][:, q * 16:(q + 1) * 16, :]
            nc.vector.tensor_mul(sl, sl, swbc)
            eng = nc.gpsimd if b == 0 else nc.vector
            eng.tensor_tensor(
                sl, sl, xp[b][:, 1 + q * 16:17 + q * 16, 1:W + 1], ALU.add)
            nc.scalar.dma_start(out_d[b, :, q * 16:(q + 1) * 16, :], sl)

    # sample 0 conv
    for sup in range(16):
        conv_strip(0, sup)
    se_block(0)
    # sample 1 conv with sample-0 stats interleaved
    for sup in range(16):
        conv_strip(1, sup)
        if sup % 2 == 1 and sup < 8:
            stats_chunk(0, sup // 2)
        if sup == 9:
            banded_final(0)
    se_block(1)
    for c in range(4):
        stats_chunk(1, c)
    banded_final(1)


def _host_prep(inp):
    import ml_dtypes
    experts = np.ascontiguousarray(inp["experts"], dtype=np.float32)
    ew = experts.reshape(E, 16, 8, CI, 9)          # [e, og, o', i, k]
    ew = ew.transpose(1, 2, 0, 4, 3)               # [og, o', e, k, i]
    ew = np.ascontiguousarray(ew).reshape(4, 4, 128, 9, 128)
    ew = np.ascontiguousarray(ew.transpose(0, 2, 1, 3, 4))  # [g4, p, 4, k, i]

    bm = np.zeros((8, 16, 8), dtype=np.float32)
    for j in range(8):
        bm[j, :, j] = 1.0
    bm = bm.reshape(128, 8)

    saw = np.asarray(inp["sa_w"], np.float32).reshape(2, 7, 7)
    Ms = np.zeros((7, 128, 128), dtype=np.float32)
    Mm = np.zeros((7, 128, 128), dtype=np.float32)
    hp = np.arange(128)
    for dh in range(7):
        for dw in range(7):
            src = hp + dh - 3
            v = (src >= 0) & (src < 128)
            Ms[dw, src[v], hp[v]] += saw[0, dh, dw] / CO
            src2 = hp + dw - 3
            v2 = (src2 >= 0) & (src2 < 128)
            Mm[dh, src2[v2], hp[v2]] += saw[1, dh, dw]
    msum = np.ascontiguousarray(Ms.transpose(1, 0, 2)).reshape(128, 896)
    mmax = np.ascontiguousarray(Mm.transpose(1, 0, 2)).reshape(128, 896)

    # packf [128, 330] f32
    pkf = np.zeros((128, 330), dtype=np.float32)
    pkf[:, 0:16] = inp["rw1"].T
    pkf[:, 16:32] = inp["rw3"].T
    pkf[:, 32:48] = inp["ca_w1"].T
    pkf[:, 48] = -np.asarray(inp["rbn2_g"], np.float32) * BNS
    pkf[:, 49] = -np.asarray(inp["rbn2_b"], np.float32)
    pkf[:, 50] = -np.asarray(inp["ca_bn2_g"], np.float32) * BNS
    pkf[:, 51] = -np.asarray(inp["ca_bn2_b"], np.float32)
    pkf[:, 52] = -float(inp["sa_bn_g"][0]) * BNS
    pkf[:, 53] = -float(inp["sa_bn_b"][0])
    pkf[0:16, 54:182] = inp["rw2"].T
    pkf[0:16, 182:310] = inp["ca_w2"].T
    pkf[0:16, 310] = np.asarray(inp["rbn1_g"], np.float32) * (BNS / HW)
    pkf[0:16, 311] = np.asarray(inp["rbn1_b"], np.float32)
    pkf[0:16, 312] = np.asarray(inp["ca_bn1_g"], np.float32) * (BNS / HW)
    pkf[0:16, 313] = np.asarray(inp["ca_bn1_b"], np.float32)
    pkf[0, 314:330] = np.asarray(inp["rb3"], np.float32)

    # packc [128, 1928] bf16
    pkc = np.zeros((128, 1928), dtype=np.float32)
    pkc[:, 0:896] = msum
    pkc[:, 896:1792] = mmax
    pkc[:, 1792:1920] = np.eye(128, dtype=np.float32)
    pkc[:, 1920:1928] = bm

    x = np.asarray(inp["x"], np.float32)
    xpad = np.zeros((B, CI, H + 2, W + 2), dtype=ml_dtypes.bfloat16)
    xpad[:, :, 1:H + 1, 1:W + 1] = x

    shared = {
        "experts_w": ew.astype(ml_dtypes.bfloat16),
        "packf": pkf,
        "packc": pkc.astype(ml_dtypes.bfloat16),
    }
    in_maps = []
    for c in range(NCORES):
        m = dict(shared)
        m["xpad"] = np.ascontiguousarray(xpad[BL * c:BL * (c + 1)])
        in_maps.append(m)
    return in_maps


def get_module():
    if "nc" not in _CACHE:
        _CACHE["nc"] = _build_module()
    return _CACHE["nc"]


def kernel(**inputs):
    nc = get_module()
    in_maps = _host_prep(inputs)
    res = run_bass_kernel_spmd(nc, in_maps, core_ids=list(range(NCORES)))
    out = np.concatenate([r["out"] for r in res.results], axis=0)
    return out.astype(np.float32)
